# revision 24
# baseline (speedup 1.0000x reference)
"""Bidirectional LSTM (B=64, T=256, D=512, U=500) on 8 Trainium2 NeuronCores.

Sharding: 2 directions x 4 batch-groups -> 16 samples per core, one direction
per core. Backward cores receive time-reversed x from the host, so the device
program is pure SPMD (identical on all 8 cores).

Per-core program:
  Phase 1 (GEMM): xz[t*16+b, 4U] = x @ Wk + b     (f32r matmuls, K=512, M=4096, N=2000)
  Phase 2 (recurrence), 256 steps. Gate banks are host-permuted to [f,g,i,o]:
      PSUM bank n accumulates xz (via identity-matmul) + h @ Wr chunks
      sig/tanh read PSUM directly on ScalarE (order: f, g, i, o)
      t2 = f*c (GpSimd), t1 = i*g, c = t1 + t2 (Vector)
      tail in transposed space: o and c are PE-transposed, tanh(cT) runs
      PSUM->SBUF at 125-partition layout, hT = oT * tanh(cT) -> feeds the
      next matmul directly; y is stored transposed and fixed up on host.
"""

import numpy as np

B, T, D, U = 64, 256, 512, 500
G4 = 4 * U            # 2000
NCORES = 8
BC = B // 4           # 16 samples per core
KCH, KQ = 4, 125      # U = 4 chunks of 125 (recurrent contraction)
DCH = 4               # D = 4 chunks of 128 (input contraction)
NSL = 500             # gate-slice / PSUM-bank width (<=512 fp32)
MT = (T * BC) // 128  # 32 M-tiles of 128 rows in the input GEMM

_CACHE = {}


def _build_program(steps=T):
    import concourse.bass as bass
    import concourse.bacc as bacc
    import concourse.tile as tile
    import concourse.mybir as mybir
    from concourse.masks import make_identity

    dt = mybir.dt
    AF = mybir.ActivationFunctionType
    f32 = dt.float32
    f32r = dt.float32r

    nc = bacc.Bacc("TRN2")

    xT = nc.dram_tensor("xT", [D, T * BC], f32r, kind="ExternalInput")  # (d, t*16+b)
    h0 = nc.dram_tensor("h0", [BC, U], f32, kind="ExternalInput")
    c0 = nc.dram_tensor("c0", [BC, U], f32r, kind="ExternalInput")
    Wk = nc.dram_tensor("Wk", [D, G4], f32r, kind="ExternalInput")   # cols [f,g,i,o]
    Wr = nc.dram_tensor("Wr", [U, G4], f32r, kind="ExternalInput")   # cols [f,g,i,o]
    bv = nc.dram_tensor("b", [G4], f32, kind="ExternalInput")
    # transposed output halves: yTa[t, q, k, b] = h_t[b, 125*k + q] (k=0,1),
    # yTb same for k=2,3
    yTa = nc.dram_tensor("yTa", [T, KQ, 2, BC], f32r, kind="ExternalOutput")
    yTb = nc.dram_tensor("yTb", [T, KQ, 2, BC], f32r, kind="ExternalOutput")
    xz = nc.dram_tensor("xzbuf", [T * BC, G4], f32r)

    with tile.TileContext(nc) as tc:
        with tc.tile_pool(name="persist", bufs=1) as persist:
            # Wr chunks stay resident: chunk k = Wr[125k:125k+125, :]
            wr_sb = persist.tile([KQ, KCH, G4], f32r)
            for k in range(KCH):
                nc.gpsimd.dma_start(wr_sb[:, k, :], Wr[k * KQ:(k + 1) * KQ, :])
            ident_f = persist.tile([BC, BC], f32)
            make_identity(nc, ident_f)
            ident = persist.tile([BC, BC], f32r)
            nc.vector.tensor_copy(ident, ident_f)

            # ---------------- Phase 1: xz = x @ Wk + b ----------------
            with tc.tile_pool(name="gx", bufs=1) as gx, \
                 tc.tile_pool(name="gpsum", bufs=2, space="PSUM") as gps, \
                 tc.tile_pool(name="gout", bufs=3) as gout:
                xT_sb = gx.tile([128, DCH, T * BC], f32r)
                wk_sb = gx.tile([128, DCH, G4], f32r)
                for k in range(DCH):
                    nc.gpsimd.dma_start(xT_sb[:, k, :], xT[k * 128:(k + 1) * 128, :])
                    nc.gpsimd.dma_start(wk_sb[:, k, :], Wk[k * 128:(k + 1) * 128, :])
                b_bc = gx.tile([128, G4], f32)
                bva = bv[:]
                nc.gpsimd.dma_start(
                    b_bc, bass.AP(bva.tensor, bva.offset, [[0, 128], [1, G4]])
                )
                for m in range(MT):
                    ps = gps.tile([128, 4, 512], f32)
                    for n in range(4):
                        for k in range(DCH):
                            nc.tensor.matmul(
                                ps[:, n, 0:NSL],
                                lhsT=xT_sb[:, k, m * 128:(m + 1) * 128],
                                rhs=wk_sb[:, k, n * NSL:(n + 1) * NSL],
                                start=(k == 0),
                                stop=(k == DCH - 1),
                            )
                    so = gout.tile([128, G4], f32r)
                    for n in range(4):
                        nc.vector.tensor_add(
                            so[:, n * NSL:(n + 1) * NSL],
                            ps[:, n, 0:NSL],
                            b_bc[:, n * NSL:(n + 1) * NSL],
                        )
                    nc.sync.dma_start(xz[m * 128:(m + 1) * 128, :], so)

            # ---------------- Phase 2: recurrence ----------------
            # Gate banks (host-permuted): 0=f 1=g 2=i 3=o.  The i bank, c
            # state, cT transpose, tanh(cT) and hT are split into unit-halves
            # A = [0,250) and B = [250,500) so each half pipelines through
            # ScalarE/VectorE/PE independently (separate PSUM banks per half
            # dodge both tile-granular deps and PSUM bank collisions).
            UH = U // 2
            with tc.tile_pool(name="state", bufs=2) as st, \
                 tc.tile_pool(name="gates", bufs=2) as gt, \
                 tc.tile_pool(name="xzin", bufs=4) as xzp, \
                 tc.tile_pool(name="zf", bufs=1, space="PSUM") as pf, \
                 tc.tile_pool(name="zg", bufs=1, space="PSUM") as pg, \
                 tc.tile_pool(name="zia", bufs=1, space="PSUM") as pia, \
                 tc.tile_pool(name="zib", bufs=1, space="PSUM") as pib, \
                 tc.tile_pool(name="zo", bufs=1, space="PSUM") as po, \
                 tc.tile_pool(name="tpo", bufs=1, space="PSUM") as tpo_p, \
                 tc.tile_pool(name="tpca", bufs=1, space="PSUM") as tpca_p, \
                 tc.tile_pool(name="tpcb", bufs=1, space="PSUM") as tpcb_p:

                cA = st.tile([BC, UH], f32r, tag="cA")
                cB = st.tile([BC, UH], f32r, tag="cB")
                nc.sync.dma_start(cA, c0[:, 0:UH])
                nc.sync.dma_start(cB, c0[:, UH:U])
                h0t = st.tile([BC, U], f32, tag="h0t")
                nc.sync.dma_start(h0t, h0[:, :])
                h_prev = st.tile([BC, U], f32r, tag="h0r")
                nc.vector.tensor_copy(h_prev, h0t)

                # initial hT halves from h0
                tpt0 = tpo_p.tile([KQ, KCH, BC], f32, tag="tpo")
                for q in range(KCH):
                    nc.tensor.matmul(
                        tpt0[:, q, :], lhsT=h_prev[:, q * KQ:(q + 1) * KQ],
                        rhs=ident, start=True, stop=True)
                hTa = st.tile([KQ, 2, BC], f32r, tag="hTa")
                hTb = st.tile([KQ, 2, BC], f32r, tag="hTb")
                nc.vector.tensor_copy(hTa, tpt0[:, 0:2, :])
                nc.vector.tensor_copy(hTb, tpt0[:, 2:4, :])

                # step-0 xz load + accumulate into fresh PSUM tiles
                xzt = xzp.tile([BC, G4], f32r, tag="xz")
                nc.sync.dma_start(xzt, xz[0:BC, :])
                zf = pf.tile([BC, 512], f32, tag="zf")
                zg = pg.tile([BC, 512], f32, tag="zg")
                ziA = pia.tile([BC, 256], f32, tag="ziA")
                ziB = pib.tile([BC, 256], f32, tag="ziB")
                zo = po.tile([BC, 512], f32, tag="zo")

                def xz_add(z_, xzt_, lo, w):
                    nc.tensor.matmul(z_[:, 0:w], lhsT=ident,
                                     rhs=xzt_[:, lo:lo + w],
                                     start=True, stop=False)

                def xz_adds_fgi(zf_, zg_, ziA_, xzt_):
                    xz_add(zf_, xzt_, 0, NSL)
                    xz_add(zg_, xzt_, NSL, NSL)
                    xz_add(ziA_, xzt_, 2 * NSL, UH)

                xz_adds_fgi(zf, zg, ziA, xzt)
                xz_add(ziB, xzt, 2 * NSL + UH, UH)
                xz_add(zo, xzt, 3 * NSL, NSL)

                def rmm(z_, k, col, w, hTh):
                    nc.tensor.matmul(
                        z_[:, 0:w], lhsT=hTh[:, k % 2, :],
                        rhs=wr_sb[:, k, col:col + w],
                        start=False, stop=(k == KCH - 1))

                for t in range(steps):
                    # prefetch next step's xz slice
                    if t + 1 < steps:
                        xzt_n = xzp.tile([BC, G4], f32r, tag="xz")
                        nc.sync.dma_start(
                            xzt_n, xz[(t + 1) * BC:(t + 2) * BC, :])

                    # recurrent matmul burst: f, g (full), i in halves, o
                    for k in range(KCH):
                        rmm(zf, k, 0, NSL, hTa if k < 2 else hTb)
                    for k in range(KCH):
                        rmm(zg, k, NSL, NSL, hTa if k < 2 else hTb)
                    for k in range(KCH):
                        rmm(ziA, k, 2 * NSL, UH, hTa if k < 2 else hTb)
                    for k in range(KCH):
                        rmm(ziB, k, 2 * NSL + UH, UH, hTa if k < 2 else hTb)
                    for k in range(KCH):
                        rmm(zo, k, 3 * NSL, NSL, hTa if k < 2 else hTb)

                    # next step's xz accumulation. ziA/ziB go via identity
                    # matmuls on the PE (start=True). zf/zg/zo are DVE copies
                    # into PSUM (emitted at the tail): the has_written bits
                    # from this step's matmuls over the same region survive,
                    # so next step's start=False matmuls accumulate onto the
                    # copied xz values.
                    if t + 1 < steps:
                        zf_n = pf.tile([BC, 512], f32, tag="zf")
                        zg_n = pg.tile([BC, 512], f32, tag="zg")
                        ziA_n = pia.tile([BC, 256], f32, tag="ziA")
                        ziB_n = pib.tile([BC, 256], f32, tag="ziB")
                        zo_n = po.tile([BC, 512], f32, tag="zo")
                        xz_adds_fgi(zf_n, zg_n, ziA_n, xzt_n)

                    # gates (ScalarE order: f, g, iA, iB, o, tanhA, tanhB)
                    f_sb = gt.tile([BC, U], f32, tag="f")
                    nc.scalar.activation(f_sb, zf[:, 0:NSL], AF.Sigmoid)
                    t2A = gt.tile([BC, UH], f32, tag="t2A")
                    t2B = gt.tile([BC, UH], f32, tag="t2B")
                    nc.gpsimd.tensor_mul(t2A, f_sb[:, 0:UH], cA)
                    nc.gpsimd.tensor_mul(t2B, f_sb[:, UH:U], cB)
                    g_sb = gt.tile([BC, U], f32, tag="g")
                    nc.scalar.activation(g_sb, zg[:, 0:NSL], AF.Tanh)
                    iA_sb = gt.tile([BC, UH], f32, tag="iA")
                    nc.scalar.activation(iA_sb, ziA[:, 0:UH], AF.Sigmoid)
                    t1A = gt.tile([BC, UH], f32, tag="t1A")
                    nc.vector.tensor_mul(t1A, iA_sb, g_sb[:, 0:UH])
                    cA_n = st.tile([BC, UH], f32r, tag="cA")
                    nc.vector.tensor_add(cA_n, t1A, t2A)
                    iB_sb = gt.tile([BC, UH], f32, tag="iB")
                    nc.scalar.activation(iB_sb, ziB[:, 0:UH], AF.Sigmoid)
                    t1B = gt.tile([BC, UH], f32, tag="t1B")
                    nc.vector.tensor_mul(t1B, iB_sb, g_sb[:, UH:U])
                    cB_n = st.tile([BC, UH], f32r, tag="cB")
                    nc.vector.tensor_add(cB_n, t1B, t2B)
                    o_sb = gt.tile([BC, U], f32r, tag="o")
                    nc.scalar.activation(o_sb, zo[:, 0:NSL], AF.Sigmoid)

                    # transposed tail, half-pipelined:
                    # T_cA | T_o | T_cB on PE; tanh(cT half) PSUM->SBUF;
                    # hT half = oT half * tanh(cT half).
                    # Transpose-mode MMs don't count as PE-busy for the HAM
                    # clock gate, so real (identity) matmuls are interleaved
                    # through the transpose section to keep the PE at 2.4 GHz.
                    tpcA = tpca_p.tile([KQ, 2, BC], f32, tag="tpcA")
                    for q in range(2):
                        nc.tensor.matmul(
                            tpcA[:, q, :], lhsT=cA_n[:, q * KQ:(q + 1) * KQ],
                            rhs=ident, start=True, stop=True)
                    tpo = tpo_p.tile([KQ, KCH, BC], f32, tag="tpo")
                    for q in range(KCH):
                        nc.tensor.matmul(
                            tpo[:, q, :], lhsT=o_sb[:, q * KQ:(q + 1) * KQ],
                            rhs=ident, start=True, stop=True)
                    tpcB = tpcb_p.tile([KQ, 2, BC], f32, tag="tpcB")
                    for q in range(2):
                        nc.tensor.matmul(
                            tpcB[:, q, :], lhsT=cB_n[:, q * KQ:(q + 1) * KQ],
                            rhs=ident, start=True, stop=True)
                    thTa = gt.tile([KQ, 2, BC], f32r, tag="thTa")
                    nc.scalar.activation(thTa, tpcA, AF.Tanh)
                    hTa_n = st.tile([KQ, 2, BC], f32r, tag="hTa")
                    nc.vector.tensor_mul(hTa_n, tpo[:, 0:2, :], thTa)
                    thTb = gt.tile([KQ, 2, BC], f32r, tag="thTb")
                    nc.scalar.activation(thTb, tpcB, AF.Tanh)
                    hTb_n = st.tile([KQ, 2, BC], f32r, tag="hTb")
                    nc.vector.tensor_mul(hTb_n, tpo[:, 2:4, :], thTb)
                    if t + 1 < steps:
                        nc.scalar.activation(ziB_n[:, 0:UH],
                                             xzt_n[:, 2 * NSL + UH:3 * NSL],
                                             AF.Copy)
                        nc.scalar.activation(zo_n[:, 0:NSL],
                                             xzt_n[:, 3 * NSL:4 * NSL],
                                             AF.Copy)
                    nc.sync.dma_start(yTa[t], hTa_n)
                    nc.sync.dma_start(yTb[t], hTb_n)

                    cA, cB = cA_n, cB_n
                    hTa, hTb = hTa_n, hTb_n
                    if t + 1 < steps:
                        zf, zg, ziA, ziB, zo = zf_n, zg_n, ziA_n, ziB_n, zo_n
    nc.finalize()
    return nc


# Keras gate order in the weights is [i, f, g, o]; kernel wants [f, g, i, o].
_PERM = np.concatenate([
    np.arange(U, 2 * U),      # f
    np.arange(2 * U, 3 * U),  # g
    np.arange(0, U),          # i
    np.arange(3 * U, 4 * U),  # o
])


def _make_in_maps(x, h_f, c_f, h_b, c_b, Wk_f, Wr_f, b_f, Wk_b, Wr_b, b_b):
    x = np.ascontiguousarray(np.asarray(x, np.float32))
    Wks = [np.ascontiguousarray(np.asarray(Wk_f, np.float32)[:, _PERM]),
           np.ascontiguousarray(np.asarray(Wk_b, np.float32)[:, _PERM])]
    Wrs = [np.ascontiguousarray(np.asarray(Wr_f, np.float32)[:, _PERM]),
           np.ascontiguousarray(np.asarray(Wr_b, np.float32)[:, _PERM])]
    bs = [np.ascontiguousarray(np.asarray(b_f, np.float32)[_PERM]),
          np.ascontiguousarray(np.asarray(b_b, np.float32)[_PERM])]
    in_maps = []
    for core in range(NCORES):
        d = core // 4           # 0 = forward, 1 = backward
        g = core % 4
        bsl = slice(g * BC, (g + 1) * BC)
        xc = x[bsl] if d == 0 else x[bsl, ::-1]
        # xT[d, t*16+b] = xc[b, t, d]
        xTc = np.ascontiguousarray(xc.transpose(2, 1, 0).reshape(D, T * BC))
        in_maps.append({
            "xT": xTc,
            "h0": np.ascontiguousarray((h_f if d == 0 else h_b)[bsl], np.float32),
            "c0": np.ascontiguousarray((c_f if d == 0 else c_b)[bsl], np.float32),
            "Wk": Wks[d],
            "Wr": Wrs[d],
            "b": bs[d],
        })
    return in_maps


def kernel(x, h_f, c_f, h_b, c_b, Wk_f, Wr_f, b_f, Wk_b, Wr_b, b_b):
    from concourse.bass_utils import run_bass_kernel_spmd

    if "nc" not in _CACHE:
        _CACHE["nc"] = _build_program()
    nc = _CACHE["nc"]
    in_maps = _make_in_maps(x, h_f, c_f, h_b, c_b, Wk_f, Wr_f, b_f, Wk_b, Wr_b, b_b)

    import os
    trace = os.environ.get("BLSTM_TRACE") == "1"
    tmpdir = os.environ.get("BLSTM_TRACE_DIR") or None
    br = run_bass_kernel_spmd(nc, in_maps, list(range(NCORES)), trace=trace, tmpdir=tmpdir)
    _CACHE["exec_time_ns"] = br.exec_time_ns
    _CACHE["br"] = br
    res = br.results

    out = np.empty((B, T, 2 * U), np.float32)
    for core in range(NCORES):
        d = core // 4
        g = core % 4
        yc = np.concatenate([res[core]["yTa"], res[core]["yTb"]], axis=2)
        # yc[t, q, k, b] = h_t[b, 125*k + q] -> [BC, T, U]
        yc = np.ascontiguousarray(np.transpose(yc, (3, 0, 2, 1))).reshape(BC, T, U)
        bsl = slice(g * BC, (g + 1) * BC)
        if d == 0:
            out[bsl, :, :U] = yc
        else:
            out[bsl, :, U:] = yc[:, ::-1]
    return out


# revision 25
# speedup vs baseline: 1.3210x; 1.3210x over previous
"""Bidirectional LSTM (B=64, T=256, D=512, U=500) on 8 Trainium2 NeuronCores.

Sharding: 2 directions x 4 batch-groups -> 16 samples per core, one direction
per core. Backward cores receive time-reversed x from the host, so the device
program is pure SPMD (identical on all 8 cores).

Per-core program:
  Phase 1 (GEMM): xz[t*16+b, 4U] = x @ Wk + b     (f32r matmuls, K=512, M=4096, N=2000)
  Phase 2 (recurrence), 256 steps. Gate banks are host-permuted to [f,g,i,o]:
      PSUM bank n accumulates xz (via identity-matmul) + h @ Wr chunks
      sig/tanh read PSUM directly on ScalarE (order: f, g, i, o)
      t2 = f*c (GpSimd), t1 = i*g, c = t1 + t2 (Vector)
      tail in transposed space: o and c are PE-transposed, tanh(cT) runs
      PSUM->SBUF at 125-partition layout, hT = oT * tanh(cT) -> feeds the
      next matmul directly; y is stored transposed and fixed up on host.
"""

import numpy as np

B, T, D, U = 64, 256, 512, 500
G4 = 4 * U            # 2000
NCORES = 8
BC = B // 4           # 16 samples per core
KCH, KQ = 4, 125      # U = 4 chunks of 125 (recurrent contraction)
DCH = 4               # D = 4 chunks of 128 (input contraction)
NSL = 500             # gate-slice / PSUM-bank width (<=512 fp32)
MT = (T * BC) // 128  # 32 M-tiles of 128 rows in the input GEMM

_CACHE = {}


def _build_program(steps=T):
    import concourse.bass as bass
    import concourse.bacc as bacc
    import concourse.tile as tile
    import concourse.mybir as mybir
    from concourse.masks import make_identity

    dt = mybir.dt
    AF = mybir.ActivationFunctionType
    f32 = dt.float32
    f32r = dt.float32r

    nc = bacc.Bacc("TRN2")

    xT = nc.dram_tensor("xT", [D, T * BC], f32r, kind="ExternalInput")  # (d, t*16+b)
    h0 = nc.dram_tensor("h0", [BC, U], f32, kind="ExternalInput")
    c0 = nc.dram_tensor("c0", [BC, U], f32r, kind="ExternalInput")
    Wk = nc.dram_tensor("Wk", [D, G4], f32r, kind="ExternalInput")   # cols [f,g,i,o]
    Wr = nc.dram_tensor("Wr", [U, G4], f32r, kind="ExternalInput")   # cols [f,g,i,o]
    bv = nc.dram_tensor("b", [G4], f32, kind="ExternalInput")
    # transposed output halves: yTa[t, q, k, b] = h_t[b, 125*k + q] (k=0,1),
    # yTb same for k=2,3
    yTa = nc.dram_tensor("yTa", [T, KQ, 2, BC], f32r, kind="ExternalOutput")
    yTb = nc.dram_tensor("yTb", [T, KQ, 2, BC], f32r, kind="ExternalOutput")
    xz = nc.dram_tensor("xzbuf", [T * BC, G4], f32r)

    with tile.TileContext(nc) as tc:
        with tc.tile_pool(name="persist", bufs=1) as persist:
            # Wr chunks stay resident: chunk k = Wr[125k:125k+125, :]
            wr_sb = persist.tile([KQ, KCH, G4], f32r)
            for k in range(KCH):
                nc.gpsimd.dma_start(wr_sb[:, k, :], Wr[k * KQ:(k + 1) * KQ, :])
            ident_f = persist.tile([BC, BC], f32)
            make_identity(nc, ident_f)
            ident = persist.tile([BC, BC], f32r)
            nc.vector.tensor_copy(ident, ident_f)

            # ---------------- Phase 1: xz = x @ Wk + b ----------------
            with tc.tile_pool(name="gx", bufs=1) as gx, \
                 tc.tile_pool(name="gpsum", bufs=2, space="PSUM") as gps, \
                 tc.tile_pool(name="gout", bufs=3) as gout:
                xT_sb = gx.tile([128, DCH, T * BC], f32r)
                wk_sb = gx.tile([128, DCH, G4], f32r)
                for k in range(DCH):
                    nc.gpsimd.dma_start(xT_sb[:, k, :], xT[k * 128:(k + 1) * 128, :])
                    nc.gpsimd.dma_start(wk_sb[:, k, :], Wk[k * 128:(k + 1) * 128, :])
                b_bc = gx.tile([128, G4], f32)
                bva = bv[:]
                nc.gpsimd.dma_start(
                    b_bc, bass.AP(bva.tensor, bva.offset, [[0, 128], [1, G4]])
                )
                for m in range(MT):
                    ps = gps.tile([128, 4, 512], f32)
                    for n in range(4):
                        for k in range(DCH):
                            nc.tensor.matmul(
                                ps[:, n, 0:NSL],
                                lhsT=xT_sb[:, k, m * 128:(m + 1) * 128],
                                rhs=wk_sb[:, k, n * NSL:(n + 1) * NSL],
                                start=(k == 0),
                                stop=(k == DCH - 1),
                            )
                    so = gout.tile([128, G4], f32r)
                    for n in range(4):
                        nc.vector.tensor_add(
                            so[:, n * NSL:(n + 1) * NSL],
                            ps[:, n, 0:NSL],
                            b_bc[:, n * NSL:(n + 1) * NSL],
                        )
                    nc.sync.dma_start(xz[m * 128:(m + 1) * 128, :], so)

            # ---------------- Phase 2: recurrence ----------------
            # Gate banks (host-permuted): 0=f 1=g 2=i 3=o.  The i bank, c
            # state, cT transpose, tanh(cT) and hT are split into unit-halves
            # A = [0,250) and B = [250,500) so each half pipelines through
            # ScalarE/VectorE/PE independently (separate PSUM banks per half
            # dodge both tile-granular deps and PSUM bank collisions).
            UH = U // 2
            with tc.tile_pool(name="state", bufs=2) as st, \
                 tc.tile_pool(name="gates", bufs=2) as gt, \
                 tc.tile_pool(name="xzin", bufs=4) as xzp, \
                 tc.tile_pool(name="zf", bufs=1, space="PSUM") as pf, \
                 tc.tile_pool(name="zg", bufs=1, space="PSUM") as pg, \
                 tc.tile_pool(name="zia", bufs=1, space="PSUM") as pia, \
                 tc.tile_pool(name="zib", bufs=1, space="PSUM") as pib, \
                 tc.tile_pool(name="zo", bufs=1, space="PSUM") as po, \
                 tc.tile_pool(name="tpo", bufs=1, space="PSUM") as tpo_p, \
                 tc.tile_pool(name="tpca", bufs=1, space="PSUM") as tpca_p, \
                 tc.tile_pool(name="tpcb", bufs=1, space="PSUM") as tpcb_p:

                cA = st.tile([BC, UH], f32r, tag="cA")
                cB = st.tile([BC, UH], f32r, tag="cB")
                nc.sync.dma_start(cA, c0[:, 0:UH])
                nc.sync.dma_start(cB, c0[:, UH:U])
                h0t = st.tile([BC, U], f32, tag="h0t")
                nc.sync.dma_start(h0t, h0[:, :])
                h_prev = st.tile([BC, U], f32r, tag="h0r")
                nc.vector.tensor_copy(h_prev, h0t)

                # initial hT halves from h0
                tpt0 = tpo_p.tile([KQ, KCH, BC], f32, tag="tpo")
                for q in range(KCH):
                    nc.tensor.matmul(
                        tpt0[:, q, :], lhsT=h_prev[:, q * KQ:(q + 1) * KQ],
                        rhs=ident, start=True, stop=True)
                hTa = st.tile([KQ, 2, BC], f32r, tag="hTa")
                hTb = st.tile([KQ, 2, BC], f32r, tag="hTb")
                nc.vector.tensor_copy(hTa, tpt0[:, 0:2, :])
                nc.vector.tensor_copy(hTb, tpt0[:, 2:4, :])

                # step-0 xz load + accumulate into fresh PSUM tiles
                xzt = xzp.tile([BC, G4], f32r, tag="xz")
                nc.sync.dma_start(xzt, xz[0:BC, :])
                zf = pf.tile([BC, 512], f32, tag="zf")
                zg = pg.tile([BC, 512], f32, tag="zg")
                ziA = pia.tile([BC, 256], f32, tag="ziA")
                ziB = pib.tile([BC, 256], f32, tag="ziB")
                zo = po.tile([BC, 512], f32, tag="zo")

                def xz_add(z_, xzt_, lo, w):
                    nc.tensor.matmul(z_[:, 0:w], lhsT=ident,
                                     rhs=xzt_[:, lo:lo + w],
                                     start=True, stop=False)

                def xz_adds_fgi(zf_, zg_, ziA_, xzt_):
                    xz_add(zf_, xzt_, 0, NSL)
                    xz_add(zg_, xzt_, NSL, NSL)
                    xz_add(ziA_, xzt_, 2 * NSL, UH)

                xz_adds_fgi(zf, zg, ziA, xzt)
                xz_add(ziB, xzt, 2 * NSL + UH, UH)
                xz_add(zo, xzt, 3 * NSL, NSL)

                def rmm(z_, k, col, w, hTh):
                    nc.tensor.matmul(
                        z_[:, 0:w], lhsT=hTh[:, k % 2, :],
                        rhs=wr_sb[:, k, col:col + w],
                        start=False, stop=(k == KCH - 1))

                for t in range(steps):
                    # prefetch next step's xz slice
                    if t + 1 < steps:
                        xzt_n = xzp.tile([BC, G4], f32r, tag="xz")
                        nc.sync.dma_start(
                            xzt_n, xz[(t + 1) * BC:(t + 2) * BC, :])

                    # recurrent matmul burst: f, g (full), i in halves, o
                    for k in range(KCH):
                        rmm(zf, k, 0, NSL, hTa if k < 2 else hTb)
                    for k in range(KCH):
                        rmm(zg, k, NSL, NSL, hTa if k < 2 else hTb)
                    for k in range(KCH):
                        rmm(ziA, k, 2 * NSL, UH, hTa if k < 2 else hTb)
                    for k in range(KCH):
                        rmm(ziB, k, 2 * NSL + UH, UH, hTa if k < 2 else hTb)
                    for k in range(KCH):
                        rmm(zo, k, 3 * NSL, NSL, hTa if k < 2 else hTb)

                    # next step's xz accumulation. ziA/ziB go via identity
                    # matmuls on the PE (start=True). zf/zg/zo are DVE copies
                    # into PSUM (emitted at the tail): the has_written bits
                    # from this step's matmuls over the same region survive,
                    # so next step's start=False matmuls accumulate onto the
                    # copied xz values.
                    if t + 1 < steps:
                        zf_n = pf.tile([BC, 512], f32, tag="zf")
                        zg_n = pg.tile([BC, 512], f32, tag="zg")
                        ziA_n = pia.tile([BC, 256], f32, tag="ziA")
                        ziB_n = pib.tile([BC, 256], f32, tag="ziB")
                        zo_n = po.tile([BC, 512], f32, tag="zo")
                        xz_adds_fgi(zf_n, zg_n, ziA_n, xzt_n)

                    # gates (ScalarE order: f, g, iA, iB, o, tanhA, tanhB)
                    f_sb = gt.tile([BC, U], f32, tag="f")
                    nc.scalar.activation(f_sb, zf[:, 0:NSL], AF.Sigmoid)
                    t2A = gt.tile([BC, UH], f32, tag="t2A")
                    t2B = gt.tile([BC, UH], f32, tag="t2B")
                    nc.gpsimd.tensor_mul(t2A, f_sb[:, 0:UH], cA)
                    nc.gpsimd.tensor_mul(t2B, f_sb[:, UH:U], cB)
                    g_sb = gt.tile([BC, U], f32, tag="g")
                    nc.scalar.activation(g_sb, zg[:, 0:NSL], AF.Tanh)
                    iA_sb = gt.tile([BC, UH], f32, tag="iA")
                    nc.scalar.activation(iA_sb, ziA[:, 0:UH], AF.Sigmoid)
                    t1A = gt.tile([BC, UH], f32, tag="t1A")
                    nc.vector.tensor_mul(t1A, iA_sb, g_sb[:, 0:UH])
                    cA_n = st.tile([BC, UH], f32r, tag="cA")
                    nc.vector.tensor_add(cA_n, t1A, t2A)
                    iB_sb = gt.tile([BC, UH], f32, tag="iB")
                    nc.scalar.activation(iB_sb, ziB[:, 0:UH], AF.Sigmoid)
                    t1B = gt.tile([BC, UH], f32, tag="t1B")
                    nc.vector.tensor_mul(t1B, iB_sb, g_sb[:, UH:U])
                    cB_n = st.tile([BC, UH], f32r, tag="cB")
                    nc.vector.tensor_add(cB_n, t1B, t2B)
                    o_sb = gt.tile([BC, U], f32r, tag="o")
                    nc.scalar.activation(o_sb, zo[:, 0:NSL], AF.Sigmoid)

                    # transposed tail, half-pipelined:
                    # T_cA | T_o | T_cB on PE; tanh(cT half) PSUM->SBUF;
                    # hT half = oT half * tanh(cT half).
                    # Transpose-mode MMs don't count as PE-busy for the HAM
                    # clock gate, so real (identity) matmuls are interleaved
                    # through the transpose section to keep the PE at 2.4 GHz.
                    tpcA = tpca_p.tile([KQ, 2, BC], f32, tag="tpcA")
                    for q in range(2):
                        nc.tensor.matmul(
                            tpcA[:, q, :], lhsT=cA_n[:, q * KQ:(q + 1) * KQ],
                            rhs=ident, start=True, stop=True)
                    if t + 1 < steps:
                        xz_add(ziB_n, xzt_n, 2 * NSL + UH, UH)
                    tpo = tpo_p.tile([KQ, KCH, BC], f32, tag="tpo")
                    for q in range(KCH):
                        nc.tensor.matmul(
                            tpo[:, q, :], lhsT=o_sb[:, q * KQ:(q + 1) * KQ],
                            rhs=ident, start=True, stop=True)
                    tpcB = tpcb_p.tile([KQ, 2, BC], f32, tag="tpcB")
                    for q in range(2):
                        nc.tensor.matmul(
                            tpcB[:, q, :], lhsT=cB_n[:, q * KQ:(q + 1) * KQ],
                            rhs=ident, start=True, stop=True)
                    if t + 1 < steps:
                        xz_add(zo_n, xzt_n, 3 * NSL, NSL)

                    thTa = gt.tile([KQ, 2, BC], f32r, tag="thTa")
                    nc.scalar.activation(thTa, tpcA, AF.Tanh)
                    hTa_n = st.tile([KQ, 2, BC], f32r, tag="hTa")
                    nc.vector.tensor_mul(hTa_n, tpo[:, 0:2, :], thTa)
                    thTb = gt.tile([KQ, 2, BC], f32r, tag="thTb")
                    nc.scalar.activation(thTb, tpcB, AF.Tanh)
                    hTb_n = st.tile([KQ, 2, BC], f32r, tag="hTb")
                    nc.vector.tensor_mul(hTb_n, tpo[:, 2:4, :], thTb)
                    nc.sync.dma_start(yTa[t], hTa_n)
                    nc.sync.dma_start(yTb[t], hTb_n)

                    cA, cB = cA_n, cB_n
                    hTa, hTb = hTa_n, hTb_n
                    if t + 1 < steps:
                        zf, zg, ziA, ziB, zo = zf_n, zg_n, ziA_n, ziB_n, zo_n
    nc.finalize()
    return nc


# Keras gate order in the weights is [i, f, g, o]; kernel wants [f, g, i, o].
_PERM = np.concatenate([
    np.arange(U, 2 * U),      # f
    np.arange(2 * U, 3 * U),  # g
    np.arange(0, U),          # i
    np.arange(3 * U, 4 * U),  # o
])


def _make_in_maps(x, h_f, c_f, h_b, c_b, Wk_f, Wr_f, b_f, Wk_b, Wr_b, b_b):
    x = np.ascontiguousarray(np.asarray(x, np.float32))
    Wks = [np.ascontiguousarray(np.asarray(Wk_f, np.float32)[:, _PERM]),
           np.ascontiguousarray(np.asarray(Wk_b, np.float32)[:, _PERM])]
    Wrs = [np.ascontiguousarray(np.asarray(Wr_f, np.float32)[:, _PERM]),
           np.ascontiguousarray(np.asarray(Wr_b, np.float32)[:, _PERM])]
    bs = [np.ascontiguousarray(np.asarray(b_f, np.float32)[_PERM]),
          np.ascontiguousarray(np.asarray(b_b, np.float32)[_PERM])]
    in_maps = []
    for core in range(NCORES):
        d = core // 4           # 0 = forward, 1 = backward
        g = core % 4
        bsl = slice(g * BC, (g + 1) * BC)
        xc = x[bsl] if d == 0 else x[bsl, ::-1]
        # xT[d, t*16+b] = xc[b, t, d]
        xTc = np.ascontiguousarray(xc.transpose(2, 1, 0).reshape(D, T * BC))
        in_maps.append({
            "xT": xTc,
            "h0": np.ascontiguousarray((h_f if d == 0 else h_b)[bsl], np.float32),
            "c0": np.ascontiguousarray((c_f if d == 0 else c_b)[bsl], np.float32),
            "Wk": Wks[d],
            "Wr": Wrs[d],
            "b": bs[d],
        })
    return in_maps


def kernel(x, h_f, c_f, h_b, c_b, Wk_f, Wr_f, b_f, Wk_b, Wr_b, b_b):
    from concourse.bass_utils import run_bass_kernel_spmd

    if "nc" not in _CACHE:
        _CACHE["nc"] = _build_program()
    nc = _CACHE["nc"]
    in_maps = _make_in_maps(x, h_f, c_f, h_b, c_b, Wk_f, Wr_f, b_f, Wk_b, Wr_b, b_b)

    import os
    trace = os.environ.get("BLSTM_TRACE") == "1"
    tmpdir = os.environ.get("BLSTM_TRACE_DIR") or None
    br = run_bass_kernel_spmd(nc, in_maps, list(range(NCORES)), trace=trace, tmpdir=tmpdir)
    _CACHE["exec_time_ns"] = br.exec_time_ns
    _CACHE["br"] = br
    res = br.results

    out = np.empty((B, T, 2 * U), np.float32)
    for core in range(NCORES):
        d = core // 4
        g = core % 4
        yc = np.concatenate([res[core]["yTa"], res[core]["yTb"]], axis=2)
        # yc[t, q, k, b] = h_t[b, 125*k + q] -> [BC, T, U]
        yc = np.ascontiguousarray(np.transpose(yc, (3, 0, 2, 1))).reshape(BC, T, U)
        bsl = slice(g * BC, (g + 1) * BC)
        if d == 0:
            out[bsl, :, :U] = yc
        else:
            out[bsl, :, U:] = yc[:, ::-1]
    return out
